# revision 44
# baseline (speedup 1.0000x reference)
"""Trainium2 Bass kernel for nn_AttentionBlock (B=32, C=512, T=1024, key=value=512).

Strategy: data-parallel over batch - each of the 8 NeuronCores processes 4
batches. Per batch, everything stays on-chip. Mixed precision tuned to the
TRN2 cost model.

Key algebraic trick: q.k = x_t^T (Wq^T Wk) x_s + beta_t + alpha_s + c0, so
the host folds Wq and Wk into one matrix M = Wq^T Wk and the device computes
a SINGLE projection g = M x instead of two (q and k). The rank-1 bias terms:
  alpha_s = (Wk^T bq).x_s + bq.bk  -> folds into the ACT exp bias column
  beta_t  = (Wq^T bk).x_t          -> host-broadcast [P,T] tensor added to
                                      the scores PSUM on the Pool engine.

Precision plan (TRN2 cost model: fp8e4 DoubleRow = 0.5 cyc/row over a
256-deep contraction; bf16 = 1 cyc/row):

  g and v projections run in fp8 DoubleRow 3-term (hi*hi + hi*lo + lo*hi)
  from host-packed fp8 hi/lo pairs of x, M, Wv.

  Scores ALSO run in fp8 DoubleRow 3-term: g is split on-device into an
  fp8 hi/lo pair (ACT copy at 1/8 scale + one fused DVE
  scalar_tensor_tensor for the residual), and the moving operand is the
  already-resident fp8 x pack. 3 cyc/column instead of bf16's 4.

  The Pool engine cannot touch PSUM, so beta enters MULTIPLICATIVELY after
  exp: w = exp(xMx*scale + alpha_col) (*) f with f_t = exp(beta_t/sqrt(d))
  host-broadcast; the multiply runs on Pool fused with the row-sum
  (scalar_tensor_tensor accum_out), replacing ACT's accum reads. The
  causal mask is a 0/1 multiply on the diagonal chunk, also on Pool.

  The out matmul runs in bf16 (1 cyc/row at any width) with exact
  128-granular causal skip, as do the exp weights w.

  A PE warmup of dummy matmuls burns the p-state ramp inside the initial
  DMA dead time.

Softmax axis is the QUERY axis (faithful to the reference): row sums Z[s]
along the free axis via activation accum_out, v rows scaled by 1/Z[s].
"""

import math
import os

import numpy as np
import ml_dtypes

os.environ.setdefault("MYCRO_LOCAL_CACHE", "1")

import concourse.bass as bass
from concourse import bacc
import concourse.tile as tile
from concourse import mybir
from concourse.bass_utils import run_bass_kernel_spmd

B, C, T = 32, 512, 1024
KEY = 512
VAL = 512
NCORES = 8
BPC = B // NCORES  # batches per core
P = 128
KT_ = KEY // P     # 4 k-tiles
ST = T // P        # 8 s-tiles
VT = VAL // P      # 4 vd-tiles
SQRT_KEY = math.sqrt(KEY)

SX = 16.0          # host pre-scale on x before fp8 split
SW = 32.0          # host pre-scale on W / M before fp8 split
RS = 16.0          # psum->fp8 rescale divisor for g (SG = SX*SW/RS = 32;
                   # device fp8e4 is IEEE e4m3, max 240 -> |SG*g| must stay low)
SG = SX * SW / RS
DESCALE = 1.0 / (SX * SW)
SCORE_SCALE = 1.0 / (SG * SX * SQRT_KEY)

F32 = mybir.dt.float32
BF16 = mybir.dt.bfloat16
F8 = mybir.dt.float8e4
F16 = mybir.dt.float16
DR = mybir.MatmulPerfMode.DoubleRow

F8NP = mybir.dt.np(F8)     # ml_dtypes.float8_e4m3

WARMUP_N = 60

HI = slice(0, 2)   # packed-level slices along the pair axis
LO = slice(2, 4)
LV = {"hi": HI, "lo": LO}

ALU = mybir.AluOpType


def score_chunks(i):
    """(t0, width, masked) chunks for s-tile i: exact 128-granular causal
    start at t=128*i, chunk widths up to 512 (PSUM bank). The causal
    triangle only touches the first 128 columns of the diagonal chunk."""
    t0 = P * i
    out = []
    masked = True
    while t0 < T:
        wd = min(512, T - t0)
        out.append((t0, wd, masked))
        masked = False
        t0 += wd
    return out


def build_program(
    psum_bufs=(3, 3, 2),
    out_copy_split=True,   # ACT for steady-state out copies, DVE for final
    warmup_n=None,
    dbg=False,
):
    if warmup_n is None:
        warmup_n = WARMUP_N
    nc = bacc.Bacc("TRN2", target_bir_lowering=False, debug=False)

    # packed fp8 hi/lo pairs for DoubleRow:
    #   xpk[b, ch, p, 2*lv + j, t] = fp8(SX * x[b, 256*ch + 128*j + p, t])
    #     split level lv: 0 = hi, 1 = residual
    #   wpk*[ch, p, 2*lv + j, k]   = fp8(SW * W[k, 256*ch + 128*j + p])
    xpk = nc.dram_tensor("xpk", [BPC, 2, P, 4, T], F8, kind="ExternalInput")
    wts_in = {}
    for w in ("m", "v"):
        wts_in[w] = nc.dram_tensor(f"wpk{w}", [2, P, 4, KEY], F8,
                                   kind="ExternalInput")
    # cst packs the 0/1 diag mask [P, 128] (bf16)
    cst = nc.dram_tensor("cst", [P, P], BF16, kind="ExternalInput")
    bvb = nc.dram_tensor("bvb", [P, VAL], F32, kind="ExternalInput")
    # abt[b][p, i] = (alpha[b, 128*i + p] + bq.bk) / sqrt(KEY)
    abt = nc.dram_tensor("abt", [BPC, P, ST], F32, kind="ExternalInput")
    # btb[b][p, t] = exp(beta[b, t] / sqrt(KEY))  (broadcast along partitions)
    btb = nc.dram_tensor("btb", [BPC, P, T], BF16, kind="ExternalInput")
    out = nc.dram_tensor("out", [BPC, VAL, T], F16, kind="ExternalOutput")
    if dbg:
        dbg_t = {
            "gh": nc.dram_tensor("dbg_gh", [P, 2, T], F8, kind="ExternalOutput"),
            "gl": nc.dram_tensor("dbg_gl", [P, 2, T], F8, kind="ExternalOutput"),
            "z": nc.dram_tensor("dbg_z", [P, ST], F32, kind="ExternalOutput"),
            "w": nc.dram_tensor("dbg_w", [P, T], BF16, kind="ExternalOutput"),
            "v": nc.dram_tensor("dbg_v", [P, VAL], BF16, kind="ExternalOutput"),
        }

    with tile.TileContext(nc) as tc:
        with (
            tc.tile_pool(name="const", bufs=1) as cpool,
            tc.tile_pool(name="xp", bufs=6) as xpool,
            tc.tile_pool(name="gp", bufs=8) as gpool,
            tc.tile_pool(name="vp", bufs=16) as vpool,
            tc.tile_pool(name="wtp", bufs=16) as wtpool,
            tc.tile_pool(name="zp", bufs=4) as zpool,
            tc.tile_pool(name="gfp", bufs=3) as gfpool,
            tc.tile_pool(name="abp", bufs=3) as abpool,
            tc.tile_pool(name="btp", bufs=3) as btpool,
            tc.tile_pool(name="osbp", bufs=8) as opool,
            tc.tile_pool(name="psa", bufs=psum_bufs[0], space="PSUM") as psum_a,
            tc.tile_pool(name="pss", bufs=psum_bufs[1], space="PSUM") as psum_s,
            tc.tile_pool(name="pso", bufs=psum_bufs[2], space="PSUM") as psum_o,
        ):
            # ---- constant tiles ----
            wsb = {}
            for w in ("m", "v"):
                for ch in range(2):
                    wsb[w, ch] = cpool.tile(
                        [P, 4, KEY], F8, tag=f"w{w}{ch}", name=f"w{w}{ch}"
                    )
            mask_sb = cpool.tile([P, P], BF16, tag="cst")
            bvb_sb = cpool.tile([P, VAL], F32, tag="bvb")

            def dma_w(w, ch):
                nc.sync.dma_start(wsb[w, ch], wts_in[w][ch])

            # per-batch tile state
            x_t = {}
            ghi_t = {}
            glo_t = {}
            v_t = {}
            wt_t = {}
            zfin_t = {}
            ztmp_t = {}
            rrec_t = {}
            ab_t = {}
            bt_t = {}

            def x_alloc(b):
                if b not in x_t:
                    x_t[b] = [
                        xpool.tile([P, 4, T], F8, tag="x", name=f"x_{b}_{ch}")
                        for ch in range(2)
                    ]

            def x_dma(b, ch, half=None, eng=None):
                x_alloc(b)
                sl = (slice(0, T) if half is None
                      else slice(half * 512, half * 512 + 512))
                (eng or nc.sync).dma_start(
                    x_t[b][ch][:, :, sl], xpk[b, ch][:, :, sl]
                )

            def ab_dma(b):
                ab_t[b] = abpool.tile([P, ST], F32, tag="ab", name=f"ab_{b}")
                bt_t[b] = btpool.tile([P, T], BF16, tag="bt", name=f"bt_{b}")
                nc.sync.dma_start(ab_t[b], abt[b])
                nc.sync.dma_start(bt_t[b], btb[b])

            def phase_x(b):
                x_dma(b, 0)
                x_dma(b, 1)
                ab_dma(b)

            TERMS = (("hi", "hi"), ("hi", "lo"), ("lo", "hi"))

            def g_alloc(b):
                if b not in ghi_t:
                    ghi_t[b] = [
                        gpool.tile([P, 2, T], F8, tag="g", name=f"gh_{b}_{m}")
                        for m in range(2)
                    ]
                    glo_t[b] = [
                        gpool.tile([P, 2, T], F8, tag="g", name=f"gl_{b}_{m}")
                        for m in range(2)
                    ]

            def g_move(b, tn, mk, ps):
                """Split one g psum group [128, 512] into fp8 hi+lo pair
                slices at 1/RS scale: hi via ACT copy, lo via one fused
                DVE (psum*1/RS - hi)."""
                m, j = mk // 2, mk % 2
                sl = slice(tn * 512, (tn + 1) * 512)
                hi = ghi_t[b][m][:, j, sl]
                lo = glo_t[b][m][:, j, sl]
                nc.scalar.activation(
                    hi, ps, mybir.ActivationFunctionType.Copy,
                    scale=1.0 / RS,
                )
                nc.vector.scalar_tensor_tensor(
                    out=lo, in0=ps, scalar=1.0 / RS, in1=hi,
                    op0=ALU.mult, op1=ALU.subtract,
                )

            def phase_g(b):
                """g = M x projection: [128c', T] tiles packed as fp8 hi/lo
                DoubleRow pairs via fp8 triple DoubleRow matmuls."""
                xs = x_t[b]
                g_alloc(b)
                for tn in range(2):
                    for mk in range(KT_):
                        ps = psum_a.tile([P, 512], F32, tag="ps_a", name="ps_a")
                        idx = 0
                        for wl, xl in TERMS:
                            for ch in range(2):
                                nc.tensor.matmul(
                                    ps,
                                    wsb["m", ch][:, LV[wl], mk * P:(mk + 1) * P],
                                    xs[ch][:, LV[xl], tn * 512:(tn + 1) * 512],
                                    start=(idx == 0),
                                    stop=(idx == 5),
                                    perf_mode=DR,
                                )
                                idx += 1
                        g_move(b, tn, mk, ps)

            def phase_v_group(b, ti):
                """One v[t, vd] tile [128t, VAL] bf16 at (SX*SW) scale; bias
                added by the DVE move (the scale folds out in the out-copy)."""
                xs = x_t[b]
                if b not in v_t:
                    v_t[b] = [
                        vpool.tile([P, VAL], BF16, tag="v", name=f"v_{b}_{i}")
                        for i in range(ST)
                    ]
                ps = psum_a.tile([P, 512], F32, tag="ps_a", name="ps_a")
                idx = 0
                for xl, wl in TERMS:
                    for ch in range(2):
                        nc.tensor.matmul(
                            ps,
                            xs[ch][:, LV[xl], ti * P:(ti + 1) * P],
                            wsb["v", ch][:, LV[wl], :],
                            start=(idx == 0),
                            stop=(idx == 5),
                            perf_mode=DR,
                        )
                        idx += 1
                nc.vector.tensor_add(out=v_t[b][ti], in0=ps, in1=bvb_sb)

            def phase_v(b):
                for ti in range(ST):
                    phase_v_group(b, ti)

            def phase_b(b, ilo=0, ihi=ST):
                """scoresT[s,t] per (s-tile, chunk) with exact causal start,
                fp8 DoubleRow from the g pair and the resident x pack; beta
                broadcast-add + causal mask on Pool; exp on ACT with
                accum_out row sums and the alpha bias column."""
                xs = x_t[b]
                if b not in wt_t:
                    wt_t[b] = [
                        wtpool.tile([P, T], BF16, tag="wt", name=f"wt_{b}_{i}")
                        for i in range(ST)
                    ]
                    zfin_t[b] = zpool.tile([P, ST], F32, tag="zfin", name="zfin")
                    ztmp_t[b] = zpool.tile([P, 2 * ST], F32, tag="ztmp", name="ztmp")
                wts = wt_t[b]
                zfin, ztmp = zfin_t[b], ztmp_t[b]
                ghi, glo = ghi_t[b], glo_t[b]

                for i in range(ilo, ihi):
                    chunks = score_chunks(i)
                    n = len(chunks)
                    for ci, (t0, wd, masked) in enumerate(chunks):
                        ps = psum_s.tile([P, 512], F32, tag="ps_s", name="ps_s")
                        idx = 0
                        for gl, xl in TERMS:
                            gt = ghi if gl == "hi" else glo
                            for m in range(2):
                                nc.tensor.matmul(
                                    ps[:, 0:wd],
                                    gt[m][:, :, i * P:(i + 1) * P],
                                    xs[m][:, LV[xl], t0:t0 + wd],
                                    start=(idx == 0),
                                    stop=(idx == 5),
                                    perf_mode=DR,
                                )
                                idx += 1
                        wsl = wts[i][:, t0:t0 + wd]
                        nc.scalar.activation(
                            wsl,
                            ps[:, 0:wd],
                            mybir.ActivationFunctionType.Exp,
                            bias=ab_t[b][:, i:i + 1],
                            scale=SCORE_SCALE,
                        )
                        if masked:
                            # zero the sub-causal triangle of the diagonal
                            # 128-block (0/1 mask, SBUF-only so Pool is ok)
                            nc.gpsimd.tensor_mul(
                                out=wts[i][:, t0:t0 + P],
                                in0=wts[i][:, t0:t0 + P],
                                in1=mask_sb,
                            )
                        zcol = (zfin[:, i:i + 1] if n == 1
                                else ztmp[:, 2 * i + ci:2 * i + ci + 1])
                        # w = e0 * f (beta factor) fused with the row sum
                        # (TensorScalarPtr is rejected on Pool, so DVE)
                        nc.vector.scalar_tensor_tensor(
                            out=wsl, in0=wsl, scalar=1.0,
                            in1=bt_t[b][:, t0:t0 + wd],
                            op0=ALU.mult, op1=ALU.mult,
                            accum_out=zcol,
                        )
                    if n > 1:
                        nc.vector.tensor_add(
                            out=zfin[:, i:i + 1],
                            in0=ztmp[:, 2 * i:2 * i + 1],
                            in1=ztmp[:, 2 * i + 1:2 * i + 2],
                        )

            def rrec_alloc(b):
                if b not in rrec_t:
                    rrec_t[b] = zpool.tile([P, ST], F32, tag="rrec", name="rrec")
                return rrec_t[b]

            def phase_c_tile(b, ti):
                """1/Z + scale v row for a single s-tile (fine-grained so out
                groups never wait on a whole half)."""
                rrec = rrec_alloc(b)
                nc.vector.reciprocal(
                    rrec[:, ti:ti + 1], zfin_t[b][:, ti:ti + 1]
                )
                nc.vector.tensor_scalar_mul(
                    v_t[b][ti], v_t[b][ti], rrec[:, ti:ti + 1]
                )

            def phase_c_half(b, h):
                rrec = rrec_alloc(b)
                nc.vector.reciprocal(
                    rrec[:, 4 * h:4 * h + 4], zfin_t[b][:, 4 * h:4 * h + 4]
                )
                for ti in range(4 * h, 4 * h + 4):
                    nc.vector.tensor_scalar_mul(
                        v_t[b][ti], v_t[b][ti], rrec[:, ti:ti + 1]
                    )

            def emit_d_group(b, c, vm, f0=0, f1=512, copy_eng="act",
                             pool=None, dma_eng=None):
                """One out-psum group for (t-chunk c, vd-tile vm), columns
                [f0,f1) of the chunk. w rows are defined-zero above their
                causal start, so full-width matmuls are valid; tiles whose
                causal start lies inside the chunk get width slices."""
                vs, wts = v_t[b], wt_t[b]
                if pool is None:
                    ps = psum_o.tile([P, 512], F32, tag="ps_o", name="ps_o")
                else:
                    ps = pool[0].tile([P, 512], F32, tag=pool[1], name=pool[1])
                t0g = 512 * c + f0
                t1g = 512 * c + f1
                scs = [sc for sc in range(4 * (c + 1)) if P * sc < t1g]
                for idx, sc in enumerate(scs):
                    lo = max(t0g, P * sc)
                    nc.tensor.matmul(
                        ps[:, lo - 512 * c:f1],
                        vs[sc][:, vm * P:(vm + 1) * P],
                        wts[sc][:, lo:t1g],
                        start=(idx == 0),
                        stop=(idx == len(scs) - 1),
                    )
                oslice = out[b, vm * P:(vm + 1) * P, t0g:t1g]
                psl = ps[:, f0:f1] if (f1 - f0) != 512 else ps
                osb = opool.tile([P, 512], F16, tag="osb", name="osb")
                osl = osb[:, 0:f1 - f0] if (f1 - f0) != 512 else osb
                if copy_eng == "act" and out_copy_split:
                    nc.scalar.activation(
                        osl, psl, mybir.ActivationFunctionType.Copy,
                        scale=DESCALE,
                    )
                else:
                    nc.vector.tensor_scalar_mul(osl, psl, DESCALE)
                (dma_eng or nc.sync).dma_start(oslice, osl)

            def phase_d_half_group(b, h, vm, steady_eng="act"):
                """Single steady-state out group (for interleaving with
                score tiles)."""
                if steady_eng == "mix":
                    eng = "dve" if vm % 2 else "act"
                else:
                    eng = steady_eng
                pool = (psum_a, "ps_a") if vm % 2 else None
                emit_d_group(b, h, vm, copy_eng=eng, pool=pool)

            def phase_d_half(b, h, final=False, steady_eng="act"):
                for vm in range(VT):
                    if final:
                        # scores done: borrow psum_s banks, alternate copy
                        # engines + DMA queues so the drain parallelizes
                        eng = "dve" if vm % 2 else "act"
                        pool = (psum_s, "ps_s") if vm % 2 else None
                        dq = nc.scalar if vm % 2 else nc.sync
                    else:
                        # steady state: ACT does the copies by default (DVE
                        # carries the f-pass and v ops); borrow idle psum_a
                        # banks so groups never wait on copy drain
                        if steady_eng == "mix":
                            eng = "dve" if vm % 2 else "act"
                        else:
                            eng = steady_eng
                        pool = (psum_a, "ps_a") if vm % 2 else None
                        dq = None
                    if final and vm == VT - 1:
                        oeng = "act" if eng == "dve" else "dve"
                        odq = nc.sync if vm % 2 else nc.scalar
                        emit_d_group(b, h, vm, 0, 256, copy_eng=eng, pool=pool,
                                     dma_eng=dq)
                        emit_d_group(b, h, vm, 256, 512, copy_eng=oeng,
                                     dma_eng=odq)
                    else:
                        emit_d_group(b, h, vm, copy_eng=eng, pool=pool,
                                     dma_eng=dq)

            def phase_cd(b):
                for h in range(2):
                    phase_c_half(b, h)
                    phase_d_half(b, h)

            # ---- startup ----
            def proj_g0_waves():
                """Whole g projection in two tn-major waves of 4 psum banks
                each (borrowed across pools -- all idle this early), ch0
                terms first within each wave. Wave A (t 0:512) starts after
                just M-ch0 + x-ch0-h0; wave B (t 512:1024) starts once the
                h1 DMAs land."""
                xs = x_t[0]
                g_alloc(0)
                terms6 = [(wl, xl, ch) for ch in range(2) for wl, xl in TERMS]
                wave_pools = {
                    0: [(psum_a, "ps_a")] * 3 + [(psum_s, "ps_s")],
                    1: [(psum_s, "ps_s")] * 2 + [(psum_o, "ps_o")] * 2,
                }
                for tn in range(2):
                    pss = {}
                    for mk in range(KT_):
                        pool, tag = wave_pools[tn][mk]
                        pss[mk] = pool.tile([P, 512], F32, tag=tag, name=tag)
                    for idx, (wl, xl, ch) in enumerate(terms6):
                        for mk in range(KT_):
                            nc.tensor.matmul(
                                pss[mk],
                                wsb["m", ch][:, LV[wl], mk * P:(mk + 1) * P],
                                xs[ch][:, LV[xl], tn * 512:(tn + 1) * 512],
                                start=(idx == 0),
                                stop=(idx == 5),
                                perf_mode=DR,
                            )
                    for mk in range(KT_):
                        g_move(0, tn, mk, pss[mk])

            def pe_warmup(n):
                """Burn the PE p-state ramp (the cost model runs matmuls at
                half clock for the first ~3us of PE activity) on dummy
                matmuls while the first DMAs are in flight."""
                warm = cpool.tile([P, 64], BF16, tag="warm", name="warm")
                nc.vector.memset(warm, 1.0)
                wps = psum_o.tile([P, 512], F32, tag="ps_o", name="ps_o")
                for _ in range(n):
                    nc.tensor.matmul(
                        wps[:, 0:64][0:64, :], warm, warm, start=True,
                        stop=True,
                    )

            def startup():
                pe_warmup(warmup_n)
                x_alloc(0)
                # split the critical startup feed across both HWDGE queues:
                # sync carries weights + x-h1, scalar carries x-h0 halves,
                # so the first matmul's two operands arrive in parallel
                dma_w("m", 0)
                x_dma(0, 0, 0, eng=nc.scalar)
                dma_w("m", 1)
                x_dma(0, 1, 0, eng=nc.scalar)
                x_dma(0, 0, 1)
                x_dma(0, 1, 1, eng=nc.scalar)
                dma_w("v", 0)
                proj_g0_waves()
                dma_w("v", 1)
                # constants are needed later (scores(0) epilogue / v bias);
                # emit after g0 so they don't compete with wv in the pipe
                ab_t[0] = abpool.tile([P, ST], F32, tag="ab", name="ab_0")
                bt_t[0] = btpool.tile([P, T], BF16, tag="bt", name="bt_0")
                nc.scalar.dma_start(mask_sb, cst[:, :])
                nc.scalar.dma_start(bvb_sb, bvb[:, :])
                nc.scalar.dma_start(ab_t[0], abt[0])
                nc.scalar.dma_start(bt_t[0], btb[0])

            startup()
            # interleave scores(0) tiles with v(0) groups so ACT's exp
            # burst overlaps the v projection instead of stalling PE.
            # Scores first: they only need g(0), while v waits on the wv
            # DMA, and a blocked 6-matmul group plugs the 4-deep PE wait
            # queue.
            for g8 in range(ST):
                phase_b(0, g8, g8 + 1)
                phase_v_group(0, g8)
            phase_c_half(0, 0)
            phase_c_half(0, 1)
            phase_x(1)
            for b in range(1, BPC):
                phase_g(b)
                phase_v(b)
                if dbg and b == 1:
                    nc.sync.dma_start(dbg_t["gh"][:], ghi_t[1][0])
                    nc.sync.dma_start(dbg_t["gl"][:], glo_t[1][0])
                    nc.sync.dma_start(dbg_t["v"][:], v_t[1][2])
                if b + 1 < BPC:
                    phase_x(b + 1)
                if b == BPC - 1:
                    # tail: interleave b-1's out groups with b's score tiles
                    # (alternating copy engines), then a progressive drain:
                    # the final half's columns [512,896) only need s-tiles
                    # 0..6, so those pieces start before s7's scores land
                    for g8 in range(8):
                        phase_d_half_group(b - 1, g8 // 4, g8 % 4,
                                           steady_eng="mix")
                        phase_b(b, g8 // 2, g8 // 2 + 1) if g8 % 2 == 0 \
                            else None
                    phase_c_half(b, 0)
                    for vm in range(VT):
                        emit_d_group(b, 0, vm, copy_eng="dve")
                        phase_b(b, 4 + vm, 5 + vm)
                        phase_c_tile(b, 4 + vm)
                    phase_d_half(b, 1, final=True)
                else:
                    # interleave b-1's 8 out groups with b's 8 score tiles:
                    # spreads ACT's exp burst across the whole PE window
                    for g8 in range(8):
                        phase_d_half_group(b - 1, g8 // 4, g8 % 4)
                        phase_b(b, g8, g8 + 1)
                    phase_c_half(b, 0)
                    phase_c_half(b, 1)
                    if dbg and b == 1:
                        nc.sync.dma_start(dbg_t["z"][:], zfin_t[1])
                        nc.sync.dma_start(dbg_t["w"][:], wt_t[1][2])

    nc.compile()
    return nc


def _split8(a, scale):
    """Split scale*a into fp8 hi + fp8 residual (both as fp8 arrays)."""
    a = np.asarray(a, dtype=np.float32) * scale
    hi = a.astype(F8NP)
    lo = (a - hi.astype(np.float32)).astype(F8NP)
    return hi, lo


def _pack_w(Wt, scale):
    """W [K, C] -> fp8 [2(ch), 128(p), 4(2*lv+j), K] with
    value[ch,p,2*lv+j,k] = fp8_lv(scale * W[k, 256*ch + 128*j + p])."""
    K = Wt.shape[0]
    hi, lo = _split8(Wt, scale)            # [K, C] fp8
    def lay(a):
        # [K, C] -> [ch, j, p, K] -> [ch, p, j, K]
        return a.T.reshape(2, 2, P, K).transpose(0, 2, 1, 3)
    hi, lo = lay(hi), lay(lo)              # [2, P, 2, K]
    pk = np.concatenate([hi, lo], axis=2)  # [2, P, 4, K]
    return np.ascontiguousarray(pk)


def _pack_x(xb, scale):
    """x [BPC, C, T] -> fp8 [BPC, 2(ch), 128(p), 4(2*lv+j), T]."""
    hi, lo = _split8(xb, scale)            # [BPC, C, T] fp8
    def lay(a):
        return a.reshape(BPC, 2, 2, P, T).transpose(0, 1, 3, 2, 4)
    pk = np.concatenate([lay(hi), lay(lo)], axis=3)  # [BPC, 2, P, 4, T]
    return np.ascontiguousarray(pk)


def _host_inputs(x, Wq, bq, Wk, bk, Wv, bv):
    x = np.asarray(x, dtype=np.float32)
    Wq = np.asarray(Wq, dtype=np.float32)
    Wk = np.asarray(Wk, dtype=np.float32)
    bq = np.asarray(bq, dtype=np.float32)
    bk = np.asarray(bk, dtype=np.float32)
    bv = np.asarray(bv, dtype=np.float32)

    M = Wq.T @ Wk                          # [C(c_t), C(c_s)]
    ua = Wk.T @ bq                         # alpha_s = ua . x_s + c0
    ub = Wq.T @ bk                         # beta_t  = ub . x_t
    c0 = float(bq @ bk)

    w8 = {
        "wpkm": _pack_w(M, SW),
        "wpkv": _pack_w(np.asarray(Wv, dtype=np.float32), SW),
    }

    r = np.arange(P)[:, None]
    u = np.arange(P)[None, :]
    maskd = np.where(u >= r, 1.0, 0.0).astype(ml_dtypes.bfloat16)
    cstv = np.ascontiguousarray(maskd)
    bvbv = np.ascontiguousarray(
        np.broadcast_to((bv * (SX * SW))[None, :], (P, VAL)).copy()
    )

    in_maps = []
    for c in range(NCORES):
        xb = x[c * BPC:(c + 1) * BPC]                       # [BPC, C, T]
        alpha = np.einsum("c,bct->bt", ua, xb) + c0         # [BPC, T]
        beta = np.einsum("c,bct->bt", ub, xb)               # [BPC, T]
        abtv = np.ascontiguousarray(
            (alpha / SQRT_KEY).reshape(BPC, ST, P).transpose(0, 2, 1)
        ).astype(np.float32)
        btbv = np.ascontiguousarray(np.broadcast_to(
            np.exp(beta / SQRT_KEY)[:, None, :], (BPC, P, T)
        )).astype(ml_dtypes.bfloat16)
        m = {
            "xpk": _pack_x(xb, SX),
            "cst": cstv, "bvb": bvbv,
            "abt": abtv, "btb": btbv,
        }
        m.update(w8)
        in_maps.append(m)
    return in_maps


_prog_cache = {}


BEST_CONFIG = {
    "psum_bufs": (3, 3, 2),
    "out_copy_split": True,
}


def _get_program():
    if "nc" not in _prog_cache:
        _prog_cache["nc"] = build_program(**BEST_CONFIG)
    return _prog_cache["nc"]


def _get_runner():
    """Build the 8-core PJRT callable once (mirrors bass2jax.run_bass_via_pjrt,
    but cached so repeat kernel() calls skip retrace/relower)."""
    if "runner" in _prog_cache:
        return _prog_cache["runner"]

    import jax
    from jax.sharding import Mesh, PartitionSpec
    from jax.experimental.shard_map import shard_map
    from concourse import mybir as _mybir
    from concourse.bass2jax import (
        _bass_exec_p,
        install_neuronx_cc_hook,
        partition_id_tensor,
    )

    install_neuronx_cc_hook()
    nc = _get_program()
    partition_name = nc.partition_id_tensor.name if nc.partition_id_tensor else None
    in_names, out_names, out_avals, zero_outs = [], [], [], []
    for alloc in nc.m.functions[0].allocations:
        if not isinstance(alloc, _mybir.MemoryLocationSet):
            continue
        name = alloc.memorylocations[0].name
        if alloc.kind == "ExternalInput":
            if name != partition_name:
                in_names.append(name)
        elif alloc.kind == "ExternalOutput":
            out_names.append(name)
            shape = tuple(alloc.tensor_shape)
            dtype = _mybir.dt.np(alloc.dtype)
            out_avals.append(jax.core.ShapedArray(shape, dtype))
            zero_outs.append(np.zeros(shape, dtype))
    n_params = len(in_names)
    all_in = list(in_names) + list(out_names)
    if partition_name is not None:
        all_in.append(partition_name)

    def _body(*args):
        operands = list(args)
        if partition_name is not None:
            operands.append(partition_id_tensor())
        outs = _bass_exec_p.bind(
            *operands,
            out_avals=tuple(out_avals),
            in_names=tuple(all_in),
            out_names=tuple(out_names),
            lowering_input_output_aliases=(),
            sim_require_finite=True,
            sim_require_nnan=True,
            nc=nc,
        )
        return tuple(outs)

    devices = jax.devices()[:NCORES]
    mesh = Mesh(np.asarray(devices), ("core",))
    n_out = len(out_names)
    fn = jax.jit(
        shard_map(
            _body,
            mesh=mesh,
            in_specs=(PartitionSpec("core"),) * (n_params + n_out),
            out_specs=(PartitionSpec("core"),) * n_out,
            check_rep=False,
        ),
        keep_unused=True,
    )
    from jax.sharding import NamedSharding

    shard = NamedSharding(mesh, PartitionSpec("core"))
    concat_zero = [
        jax.device_put(np.zeros((NCORES * z.shape[0], *z.shape[1:]), z.dtype), shard)
        for z in zero_outs
    ]
    runner = (fn, in_names, out_avals, concat_zero)
    _prog_cache["runner"] = runner
    return runner


def kernel(x, Wq, bq, Wk, bk, Wv, bv):
    in_maps = _host_inputs(x, Wq, bq, Wk, bk, Wv, bv)
    fn, in_names, out_avals, concat_zero = _get_runner()
    concat_in = [
        np.concatenate([np.asarray(in_maps[c][nm]) for c in range(NCORES)], axis=0)
        for nm in in_names
    ]
    out_arrs = fn(*concat_in, *concat_zero)
    full = np.asarray(out_arrs[0]).reshape(NCORES * BPC, *out_avals[0].shape[1:])
    return full.astype(np.float32)


# revision 51
# speedup vs baseline: 1.0056x; 1.0056x over previous
"""Trainium2 Bass kernel for nn_AttentionBlock (B=32, C=512, T=1024, key=value=512).

Strategy: data-parallel over batch - each of the 8 NeuronCores processes 4
batches. Per batch, everything stays on-chip. Mixed precision tuned to the
TRN2 cost model.

Key algebraic trick: q.k = x_t^T (Wq^T Wk) x_s + beta_t + alpha_s + c0, so
the host folds Wq and Wk into one matrix M = Wq^T Wk and the device computes
a SINGLE projection g = M x instead of two (q and k). The rank-1 bias terms:
  alpha_s = (Wk^T bq).x_s + bq.bk  -> folds into the ACT exp bias column
  beta_t  = (Wq^T bk).x_t          -> host-broadcast [P,T] tensor added to
                                      the scores PSUM on the Pool engine.

Precision plan (TRN2 cost model: fp8e4 DoubleRow = 0.5 cyc/row over a
256-deep contraction; bf16 = 1 cyc/row):

  g and v projections run in fp8 DoubleRow 3-term (hi*hi + hi*lo + lo*hi)
  from host-packed fp8 hi/lo pairs of x, M, Wv.

  Scores ALSO run in fp8 DoubleRow 3-term: g is split on-device into an
  fp8 hi/lo pair (ACT copy at 1/8 scale + one fused DVE
  scalar_tensor_tensor for the residual), and the moving operand is the
  already-resident fp8 x pack. 3 cyc/column instead of bf16's 4.

  The Pool engine cannot touch PSUM, so beta enters MULTIPLICATIVELY after
  exp: w = exp(xMx*scale + alpha_col) (*) f with f_t = exp(beta_t/sqrt(d))
  host-broadcast; the multiply runs on Pool fused with the row-sum
  (scalar_tensor_tensor accum_out), replacing ACT's accum reads. The
  causal mask is a 0/1 multiply on the diagonal chunk, also on Pool.

  The out matmul runs in bf16 (1 cyc/row at any width) with exact
  128-granular causal skip, as do the exp weights w.

  A PE warmup of dummy matmuls burns the p-state ramp inside the initial
  DMA dead time.

Softmax axis is the QUERY axis (faithful to the reference): row sums Z[s]
along the free axis via activation accum_out, v rows scaled by 1/Z[s].
"""

import math
import os

import numpy as np
import ml_dtypes

os.environ.setdefault("MYCRO_LOCAL_CACHE", "1")

import concourse.bass as bass
from concourse import bacc
import concourse.tile as tile
from concourse import mybir
from concourse.bass_utils import run_bass_kernel_spmd

B, C, T = 32, 512, 1024
KEY = 512
VAL = 512
NCORES = 8
BPC = B // NCORES  # batches per core
P = 128
KT_ = KEY // P     # 4 k-tiles
ST = T // P        # 8 s-tiles
VT = VAL // P      # 4 vd-tiles
SQRT_KEY = math.sqrt(KEY)

SX = 16.0          # host pre-scale on x before fp8 split
SW = 32.0          # host pre-scale on W / M before fp8 split
RS = 16.0          # psum->fp8 rescale divisor for g (SG = SX*SW/RS = 32;
                   # device fp8e4 is IEEE e4m3, max 240 -> |SG*g| must stay low)
SG = SX * SW / RS
DESCALE = 1.0 / (SX * SW)
SCORE_SCALE = 1.0 / (SG * SX * SQRT_KEY)

F32 = mybir.dt.float32
BF16 = mybir.dt.bfloat16
F8 = mybir.dt.float8e4
F16 = mybir.dt.float16
DR = mybir.MatmulPerfMode.DoubleRow

F8NP = mybir.dt.np(F8)     # ml_dtypes.float8_e4m3

WARMUP_N = 60

HI = slice(0, 2)   # packed-level slices along the pair axis
LO = slice(2, 4)
LV = {"hi": HI, "lo": LO}

ALU = mybir.AluOpType


def score_chunks(i):
    """(t0, width, masked) chunks for s-tile i: exact 128-granular causal
    start at t=128*i, chunk widths up to 512 (PSUM bank). The causal
    triangle only touches the first 128 columns of the diagonal chunk."""
    t0 = P * i
    out = []
    masked = True
    while t0 < T:
        wd = min(512, T - t0)
        out.append((t0, wd, masked))
        masked = False
        t0 += wd
    return out


def build_program(
    psum_bufs=(3, 3, 2),
    out_copy_split=True,   # ACT for steady-state out copies, DVE for final
    warmup_n=None,
    dbg=False,
):
    if warmup_n is None:
        warmup_n = WARMUP_N
    nc = bacc.Bacc("TRN2", target_bir_lowering=False, debug=False)

    # packed fp8 hi/lo pairs for DoubleRow:
    #   xpk[b, ch, p, 2*lv + j, t] = fp8(SX * x[b, 256*ch + 128*j + p, t])
    #     split level lv: 0 = hi, 1 = residual
    #   wpk*[ch, p, 2*lv + j, k]   = fp8(SW * W[k, 256*ch + 128*j + p])
    xpk = nc.dram_tensor("xpk", [BPC, 2, P, 4, T], F8, kind="ExternalInput")
    wts_in = {}
    for w in ("m", "v"):
        wts_in[w] = nc.dram_tensor(f"wpk{w}", [2, P, 4, KEY], F8,
                                   kind="ExternalInput")
    # cst packs the 0/1 diag mask [P, 128] (bf16)
    cst = nc.dram_tensor("cst", [P, P], BF16, kind="ExternalInput")
    bvb = nc.dram_tensor("bvb", [P, VAL], F32, kind="ExternalInput")
    # abt[b][p, i] = (alpha[b, 128*i + p] + bq.bk) / sqrt(KEY)
    abt = nc.dram_tensor("abt", [BPC, P, ST], F32, kind="ExternalInput")
    # btb[b][p, t] = exp(beta[b, t] / sqrt(KEY))  (broadcast along partitions)
    btb = nc.dram_tensor("btb", [BPC, P, T], BF16, kind="ExternalInput")
    out = nc.dram_tensor("out", [BPC, VAL, T], F16, kind="ExternalOutput")
    if dbg:
        dbg_t = {
            "gh": nc.dram_tensor("dbg_gh", [P, 2, T], F8, kind="ExternalOutput"),
            "gl": nc.dram_tensor("dbg_gl", [P, 2, T], F8, kind="ExternalOutput"),
            "z": nc.dram_tensor("dbg_z", [P, ST], F32, kind="ExternalOutput"),
            "w": nc.dram_tensor("dbg_w", [P, T], BF16, kind="ExternalOutput"),
            "v": nc.dram_tensor("dbg_v", [P, VAL], BF16, kind="ExternalOutput"),
        }

    with tile.TileContext(nc) as tc:
        with (
            tc.tile_pool(name="const", bufs=1) as cpool,
            tc.tile_pool(name="xp", bufs=6) as xpool,
            tc.tile_pool(name="gp", bufs=16) as gpool,
            tc.tile_pool(name="vp", bufs=16) as vpool,
            tc.tile_pool(name="wtp", bufs=16) as wtpool,
            tc.tile_pool(name="zp", bufs=4) as zpool,
            tc.tile_pool(name="gfp", bufs=3) as gfpool,
            tc.tile_pool(name="abp", bufs=3) as abpool,
            tc.tile_pool(name="btp", bufs=3) as btpool,
            tc.tile_pool(name="osbp", bufs=8) as opool,
            tc.tile_pool(name="psa", bufs=psum_bufs[0], space="PSUM") as psum_a,
            tc.tile_pool(name="pss", bufs=psum_bufs[1], space="PSUM") as psum_s,
            tc.tile_pool(name="pso", bufs=psum_bufs[2], space="PSUM") as psum_o,
        ):
            # ---- constant tiles ----
            wsb = {}
            for w in ("m", "v"):
                for ch in range(2):
                    wsb[w, ch] = cpool.tile(
                        [P, 4, KEY], F8, tag=f"w{w}{ch}", name=f"w{w}{ch}"
                    )
            mask_sb = cpool.tile([P, P], BF16, tag="cst")
            bvb_sb = cpool.tile([P, VAL], F32, tag="bvb")

            def dma_w(w, ch):
                nc.sync.dma_start(wsb[w, ch], wts_in[w][ch])

            # per-batch tile state
            x_t = {}
            ghi_t = {}
            glo_t = {}
            v_t = {}
            wt_t = {}
            zfin_t = {}
            ztmp_t = {}
            rrec_t = {}
            ab_t = {}
            bt_t = {}

            def x_alloc(b):
                if b not in x_t:
                    x_t[b] = [
                        xpool.tile([P, 4, T], F8, tag="x", name=f"x_{b}_{ch}")
                        for ch in range(2)
                    ]

            def x_dma(b, ch, half=None, eng=None):
                x_alloc(b)
                sl = (slice(0, T) if half is None
                      else slice(half * 512, half * 512 + 512))
                (eng or nc.sync).dma_start(
                    x_t[b][ch][:, :, sl], xpk[b, ch][:, :, sl]
                )

            def ab_dma(b):
                ab_t[b] = abpool.tile([P, ST], F32, tag="ab", name=f"ab_{b}")
                bt_t[b] = btpool.tile([P, T], BF16, tag="bt", name=f"bt_{b}")
                nc.sync.dma_start(ab_t[b], abt[b])
                nc.sync.dma_start(bt_t[b], btb[b])

            def phase_x(b):
                x_dma(b, 0)
                x_dma(b, 1)
                ab_dma(b)

            TERMS = (("hi", "hi"), ("hi", "lo"), ("lo", "hi"))

            def g_alloc(b):
                # tiles split by T-half so scores s-tiles 0..3 depend only
                # on the tn=0 projection groups (deps are tile-granular)
                if b not in ghi_t:
                    ghi_t[b] = {
                        (m, h): gpool.tile([P, 2, 512], F8, tag="g",
                                           name=f"gh_{b}_{m}{h}")
                        for m in range(2) for h in range(2)
                    }
                    glo_t[b] = {
                        (m, h): gpool.tile([P, 2, 512], F8, tag="g",
                                           name=f"gl_{b}_{m}{h}")
                        for m in range(2) for h in range(2)
                    }

            def g_move(b, tn, mk, ps):
                """Split one g psum group [128, 512] into fp8 hi+lo pair
                slices at 1/RS scale: hi via ACT copy, lo via one fused
                DVE (psum*1/RS - hi)."""
                m, j = mk // 2, mk % 2
                hi = ghi_t[b][m, tn][:, j, :]
                lo = glo_t[b][m, tn][:, j, :]
                nc.scalar.activation(
                    hi, ps, mybir.ActivationFunctionType.Copy,
                    scale=1.0 / RS,
                )
                nc.vector.scalar_tensor_tensor(
                    out=lo, in0=ps, scalar=1.0 / RS, in1=hi,
                    op0=ALU.mult, op1=ALU.subtract,
                )

            def phase_g(b):
                """g = M x projection: [128c', T] tiles packed as fp8 hi/lo
                DoubleRow pairs via fp8 triple DoubleRow matmuls."""
                xs = x_t[b]
                g_alloc(b)
                for tn in range(2):
                    for mk in range(KT_):
                        ps = psum_a.tile([P, 512], F32, tag="ps_a", name="ps_a")
                        idx = 0
                        for wl, xl in TERMS:
                            for ch in range(2):
                                nc.tensor.matmul(
                                    ps,
                                    wsb["m", ch][:, LV[wl], mk * P:(mk + 1) * P],
                                    xs[ch][:, LV[xl], tn * 512:(tn + 1) * 512],
                                    start=(idx == 0),
                                    stop=(idx == 5),
                                    perf_mode=DR,
                                )
                                idx += 1
                        g_move(b, tn, mk, ps)

            def phase_v_group(b, ti):
                """One v[t, vd] tile [128t, VAL] bf16 at (SX*SW) scale; bias
                added by the DVE move (the scale folds out in the out-copy)."""
                xs = x_t[b]
                if b not in v_t:
                    v_t[b] = [
                        vpool.tile([P, VAL], BF16, tag="v", name=f"v_{b}_{i}")
                        for i in range(ST)
                    ]
                ps = psum_a.tile([P, 512], F32, tag="ps_a", name="ps_a")
                idx = 0
                for xl, wl in TERMS:
                    for ch in range(2):
                        nc.tensor.matmul(
                            ps,
                            xs[ch][:, LV[xl], ti * P:(ti + 1) * P],
                            wsb["v", ch][:, LV[wl], :],
                            start=(idx == 0),
                            stop=(idx == 5),
                            perf_mode=DR,
                        )
                        idx += 1
                nc.vector.tensor_add(out=v_t[b][ti], in0=ps, in1=bvb_sb)

            def phase_v(b):
                for ti in range(ST):
                    phase_v_group(b, ti)

            def phase_b(b, ilo=0, ihi=ST):
                """scoresT[s,t] per (s-tile, chunk) with exact causal start,
                fp8 DoubleRow from the g pair and the resident x pack; beta
                broadcast-add + causal mask on Pool; exp on ACT with
                accum_out row sums and the alpha bias column."""
                xs = x_t[b]
                if b not in wt_t:
                    wt_t[b] = [
                        wtpool.tile([P, T], BF16, tag="wt", name=f"wt_{b}_{i}")
                        for i in range(ST)
                    ]
                    zfin_t[b] = zpool.tile([P, ST], F32, tag="zfin", name="zfin")
                    ztmp_t[b] = zpool.tile([P, 2 * ST], F32, tag="ztmp", name="ztmp")
                wts = wt_t[b]
                zfin, ztmp = zfin_t[b], ztmp_t[b]
                ghi, glo = ghi_t[b], glo_t[b]

                for i in range(ilo, ihi):
                    chunks = score_chunks(i)
                    n = len(chunks)
                    ih, io = i // 4, (i % 4) * P
                    for ci, (t0, wd, masked) in enumerate(chunks):
                        ps = psum_s.tile([P, 512], F32, tag="ps_s", name="ps_s")
                        idx = 0
                        for gl, xl in TERMS:
                            gt = ghi if gl == "hi" else glo
                            for m in range(2):
                                nc.tensor.matmul(
                                    ps[:, 0:wd],
                                    gt[m, ih][:, :, io:io + P],
                                    xs[m][:, LV[xl], t0:t0 + wd],
                                    start=(idx == 0),
                                    stop=(idx == 5),
                                    perf_mode=DR,
                                )
                                idx += 1
                        wsl = wts[i][:, t0:t0 + wd]
                        nc.scalar.activation(
                            wsl,
                            ps[:, 0:wd],
                            mybir.ActivationFunctionType.Exp,
                            bias=ab_t[b][:, i:i + 1],
                            scale=SCORE_SCALE,
                        )
                        if masked:
                            # zero the sub-causal triangle of the diagonal
                            # 128-block (0/1 mask, SBUF-only so Pool is ok)
                            nc.gpsimd.tensor_mul(
                                out=wts[i][:, t0:t0 + P],
                                in0=wts[i][:, t0:t0 + P],
                                in1=mask_sb,
                            )
                        zcol = (zfin[:, i:i + 1] if n == 1
                                else ztmp[:, 2 * i + ci:2 * i + ci + 1])
                        # w = e0 * f (beta factor) fused with the row sum
                        # (TensorScalarPtr is rejected on Pool, so DVE)
                        nc.vector.scalar_tensor_tensor(
                            out=wsl, in0=wsl, scalar=1.0,
                            in1=bt_t[b][:, t0:t0 + wd],
                            op0=ALU.mult, op1=ALU.mult,
                            accum_out=zcol,
                        )
                    if n > 1:
                        nc.vector.tensor_add(
                            out=zfin[:, i:i + 1],
                            in0=ztmp[:, 2 * i:2 * i + 1],
                            in1=ztmp[:, 2 * i + 1:2 * i + 2],
                        )

            def rrec_alloc(b):
                if b not in rrec_t:
                    rrec_t[b] = zpool.tile([P, ST], F32, tag="rrec", name="rrec")
                return rrec_t[b]

            def phase_c_tile(b, ti):
                """1/Z + scale v row for a single s-tile (fine-grained so out
                groups never wait on a whole half)."""
                rrec = rrec_alloc(b)
                nc.vector.reciprocal(
                    rrec[:, ti:ti + 1], zfin_t[b][:, ti:ti + 1]
                )
                nc.vector.tensor_scalar_mul(
                    v_t[b][ti], v_t[b][ti], rrec[:, ti:ti + 1]
                )

            def phase_c_half(b, h):
                rrec = rrec_alloc(b)
                nc.vector.reciprocal(
                    rrec[:, 4 * h:4 * h + 4], zfin_t[b][:, 4 * h:4 * h + 4]
                )
                for ti in range(4 * h, 4 * h + 4):
                    nc.vector.tensor_scalar_mul(
                        v_t[b][ti], v_t[b][ti], rrec[:, ti:ti + 1]
                    )

            def emit_d_group(b, c, vm, f0=0, f1=512, copy_eng="act",
                             pool=None, dma_eng=None):
                """One out-psum group for (t-chunk c, vd-tile vm), columns
                [f0,f1) of the chunk. w rows are defined-zero above their
                causal start, so full-width matmuls are valid; tiles whose
                causal start lies inside the chunk get width slices."""
                vs, wts = v_t[b], wt_t[b]
                if pool is None:
                    ps = psum_o.tile([P, 512], F32, tag="ps_o", name="ps_o")
                else:
                    ps = pool[0].tile([P, 512], F32, tag=pool[1], name=pool[1])
                t0g = 512 * c + f0
                t1g = 512 * c + f1
                scs = [sc for sc in range(4 * (c + 1)) if P * sc < t1g]
                for idx, sc in enumerate(scs):
                    lo = max(t0g, P * sc)
                    nc.tensor.matmul(
                        ps[:, lo - 512 * c:f1],
                        vs[sc][:, vm * P:(vm + 1) * P],
                        wts[sc][:, lo:t1g],
                        start=(idx == 0),
                        stop=(idx == len(scs) - 1),
                    )
                oslice = out[b, vm * P:(vm + 1) * P, t0g:t1g]
                psl = ps[:, f0:f1] if (f1 - f0) != 512 else ps
                osb = opool.tile([P, 512], F16, tag="osb", name="osb")
                osl = osb[:, 0:f1 - f0] if (f1 - f0) != 512 else osb
                if copy_eng == "act" and out_copy_split:
                    nc.scalar.activation(
                        osl, psl, mybir.ActivationFunctionType.Copy,
                        scale=DESCALE,
                    )
                else:
                    nc.vector.tensor_scalar_mul(osl, psl, DESCALE)
                (dma_eng or nc.sync).dma_start(oslice, osl)

            def phase_d_half_group(b, h, vm, steady_eng="act"):
                """Single steady-state out group (for interleaving with
                score tiles)."""
                if steady_eng == "mix":
                    eng = "dve" if vm % 2 else "act"
                else:
                    eng = steady_eng
                pool = (psum_a, "ps_a") if vm % 2 else None
                emit_d_group(b, h, vm, copy_eng=eng, pool=pool)

            def phase_d_half(b, h, final=False, steady_eng="act"):
                for vm in range(VT):
                    if final:
                        # scores done: borrow psum_s banks, alternate copy
                        # engines + DMA queues so the drain parallelizes
                        eng = "dve" if vm % 2 else "act"
                        pool = (psum_s, "ps_s") if vm % 2 else None
                        dq = nc.scalar if vm % 2 else nc.sync
                    else:
                        # steady state: ACT does the copies by default (DVE
                        # carries the f-pass and v ops); borrow idle psum_a
                        # banks so groups never wait on copy drain
                        if steady_eng == "mix":
                            eng = "dve" if vm % 2 else "act"
                        else:
                            eng = steady_eng
                        pool = (psum_a, "ps_a") if vm % 2 else None
                        dq = None
                    if final and vm == VT - 1:
                        oeng = "act" if eng == "dve" else "dve"
                        odq = nc.sync if vm % 2 else nc.scalar
                        emit_d_group(b, h, vm, 0, 256, copy_eng=eng, pool=pool,
                                     dma_eng=dq)
                        emit_d_group(b, h, vm, 256, 512, copy_eng=oeng,
                                     dma_eng=odq)
                    else:
                        emit_d_group(b, h, vm, copy_eng=eng, pool=pool,
                                     dma_eng=dq)

            def phase_cd(b):
                for h in range(2):
                    phase_c_half(b, h)
                    phase_d_half(b, h)

            # ---- startup ----
            def proj_g0_waves():
                """Whole g projection in two tn-major waves of 4 psum banks
                each (borrowed across pools -- all idle this early), ch0
                terms first within each wave. Wave A (t 0:512) starts after
                just M-ch0 + x-ch0-h0; wave B (t 512:1024) starts once the
                h1 DMAs land."""
                xs = x_t[0]
                g_alloc(0)
                terms6 = [(wl, xl, ch) for ch in range(2) for wl, xl in TERMS]
                wave_pools = {
                    0: [(psum_a, "ps_a")] * 3 + [(psum_s, "ps_s")],
                    1: [(psum_s, "ps_s")] * 2 + [(psum_o, "ps_o")] * 2,
                }
                for tn in range(2):
                    pss = {}
                    for mk in range(KT_):
                        pool, tag = wave_pools[tn][mk]
                        pss[mk] = pool.tile([P, 512], F32, tag=tag, name=tag)
                    for idx, (wl, xl, ch) in enumerate(terms6):
                        for mk in range(KT_):
                            nc.tensor.matmul(
                                pss[mk],
                                wsb["m", ch][:, LV[wl], mk * P:(mk + 1) * P],
                                xs[ch][:, LV[xl], tn * 512:(tn + 1) * 512],
                                start=(idx == 0),
                                stop=(idx == 5),
                                perf_mode=DR,
                            )
                    for mk in range(KT_):
                        g_move(0, tn, mk, pss[mk])

            def pe_warmup(n):
                """Burn the PE p-state ramp (the cost model runs matmuls at
                half clock for the first ~3us of PE activity) on dummy
                matmuls while the first DMAs are in flight."""
                warm = cpool.tile([P, 64], BF16, tag="warm", name="warm")
                nc.gpsimd.memset(warm, 1.0)
                wps = psum_o.tile([P, 512], F32, tag="ps_o", name="ps_o")
                for _ in range(n):
                    nc.tensor.matmul(
                        wps[:, 0:64][0:64, :], warm, warm, start=True,
                        stop=True,
                    )

            def startup():
                pe_warmup(warmup_n)
                x_alloc(0)
                # split the critical startup feed across both HWDGE queues:
                # sync carries weights + x-h1, scalar carries x-h0 halves,
                # so the first matmul's two operands arrive in parallel
                dma_w("m", 0)
                x_dma(0, 0, 0, eng=nc.scalar)
                dma_w("m", 1)
                x_dma(0, 1, 0, eng=nc.scalar)
                x_dma(0, 0, 1)
                x_dma(0, 1, 1, eng=nc.scalar)
                dma_w("v", 0)
                proj_g0_waves()
                dma_w("v", 1)
                # constants are needed later (scores(0) epilogue / v bias);
                # emit after g0 so they don't compete with wv in the pipe
                ab_t[0] = abpool.tile([P, ST], F32, tag="ab", name="ab_0")
                bt_t[0] = btpool.tile([P, T], BF16, tag="bt", name="bt_0")
                nc.scalar.dma_start(mask_sb, cst[:, :])
                nc.scalar.dma_start(bvb_sb, bvb[:, :])
                nc.scalar.dma_start(ab_t[0], abt[0])
                nc.scalar.dma_start(bt_t[0], btb[0])

            startup()
            # interleave v(0) groups with scores(0) tiles so ACT's exp
            # burst overlaps the v projection instead of stalling PE.
            # v first: wv lands before the wave-A g moves complete.
            for g8 in range(ST):
                phase_v_group(0, g8)
                phase_b(0, g8, g8 + 1)
            phase_c_half(0, 0)
            phase_c_half(0, 1)
            phase_x(1)
            for b in range(1, BPC):
                phase_g(b)
                phase_v(b)
                if dbg and b == 1:
                    for h_ in range(2):
                        sl_ = slice(h_ * 512, (h_ + 1) * 512)
                        nc.sync.dma_start(dbg_t["gh"][:, :, sl_],
                                          ghi_t[1][0, h_])
                        nc.sync.dma_start(dbg_t["gl"][:, :, sl_],
                                          glo_t[1][0, h_])
                    nc.sync.dma_start(dbg_t["v"][:], v_t[1][2])
                if b + 1 < BPC:
                    phase_x(b + 1)
                if b == BPC - 1:
                    # tail: interleave b-1's out groups with b's score tiles
                    # (alternating copy engines), then a progressive drain:
                    # the final half's columns [512,896) only need s-tiles
                    # 0..6, so those pieces start before s7's scores land
                    for g8 in range(8):
                        phase_d_half_group(b - 1, g8 // 4, g8 % 4,
                                           steady_eng="mix")
                        phase_b(b, g8 // 2, g8 // 2 + 1) if g8 % 2 == 0 \
                            else None
                    phase_c_half(b, 0)
                    for vm in range(VT):
                        emit_d_group(b, 0, vm, copy_eng="dve")
                        phase_b(b, 4 + vm, 5 + vm)
                        phase_c_tile(b, 4 + vm)
                    phase_d_half(b, 1, final=True)
                else:
                    # interleave b-1's 8 out groups with b's 8 score tiles:
                    # spreads ACT's exp burst across the whole PE window
                    for g8 in range(8):
                        phase_d_half_group(b - 1, g8 // 4, g8 % 4)
                        phase_b(b, g8, g8 + 1)
                    phase_c_half(b, 0)
                    phase_c_half(b, 1)
                    if dbg and b == 1:
                        nc.sync.dma_start(dbg_t["z"][:], zfin_t[1])
                        nc.sync.dma_start(dbg_t["w"][:], wt_t[1][2])

    nc.compile()
    return nc


def _split8(a, scale):
    """Split scale*a into fp8 hi + fp8 residual (both as fp8 arrays)."""
    a = np.asarray(a, dtype=np.float32) * scale
    hi = a.astype(F8NP)
    lo = (a - hi.astype(np.float32)).astype(F8NP)
    return hi, lo


def _pack_w(Wt, scale):
    """W [K, C] -> fp8 [2(ch), 128(p), 4(2*lv+j), K] with
    value[ch,p,2*lv+j,k] = fp8_lv(scale * W[k, 256*ch + 128*j + p])."""
    K = Wt.shape[0]
    hi, lo = _split8(Wt, scale)            # [K, C] fp8
    def lay(a):
        # [K, C] -> [ch, j, p, K] -> [ch, p, j, K]
        return a.T.reshape(2, 2, P, K).transpose(0, 2, 1, 3)
    hi, lo = lay(hi), lay(lo)              # [2, P, 2, K]
    pk = np.concatenate([hi, lo], axis=2)  # [2, P, 4, K]
    return np.ascontiguousarray(pk)


def _pack_x(xb, scale):
    """x [BPC, C, T] -> fp8 [BPC, 2(ch), 128(p), 4(2*lv+j), T]."""
    hi, lo = _split8(xb, scale)            # [BPC, C, T] fp8
    def lay(a):
        return a.reshape(BPC, 2, 2, P, T).transpose(0, 1, 3, 2, 4)
    pk = np.concatenate([lay(hi), lay(lo)], axis=3)  # [BPC, 2, P, 4, T]
    return np.ascontiguousarray(pk)


def _host_inputs(x, Wq, bq, Wk, bk, Wv, bv):
    x = np.asarray(x, dtype=np.float32)
    Wq = np.asarray(Wq, dtype=np.float32)
    Wk = np.asarray(Wk, dtype=np.float32)
    bq = np.asarray(bq, dtype=np.float32)
    bk = np.asarray(bk, dtype=np.float32)
    bv = np.asarray(bv, dtype=np.float32)

    M = Wq.T @ Wk                          # [C(c_t), C(c_s)]
    ua = Wk.T @ bq                         # alpha_s = ua . x_s + c0
    ub = Wq.T @ bk                         # beta_t  = ub . x_t
    c0 = float(bq @ bk)

    w8 = {
        "wpkm": _pack_w(M, SW),
        "wpkv": _pack_w(np.asarray(Wv, dtype=np.float32), SW),
    }

    r = np.arange(P)[:, None]
    u = np.arange(P)[None, :]
    maskd = np.where(u >= r, 1.0, 0.0).astype(ml_dtypes.bfloat16)
    cstv = np.ascontiguousarray(maskd)
    bvbv = np.ascontiguousarray(
        np.broadcast_to((bv * (SX * SW))[None, :], (P, VAL)).copy()
    )

    in_maps = []
    for c in range(NCORES):
        xb = x[c * BPC:(c + 1) * BPC]                       # [BPC, C, T]
        alpha = np.einsum("c,bct->bt", ua, xb) + c0         # [BPC, T]
        beta = np.einsum("c,bct->bt", ub, xb)               # [BPC, T]
        abtv = np.ascontiguousarray(
            (alpha / SQRT_KEY).reshape(BPC, ST, P).transpose(0, 2, 1)
        ).astype(np.float32)
        btbv = np.ascontiguousarray(np.broadcast_to(
            np.exp(beta / SQRT_KEY)[:, None, :], (BPC, P, T)
        )).astype(ml_dtypes.bfloat16)
        m = {
            "xpk": _pack_x(xb, SX),
            "cst": cstv, "bvb": bvbv,
            "abt": abtv, "btb": btbv,
        }
        m.update(w8)
        in_maps.append(m)
    return in_maps


_prog_cache = {}


BEST_CONFIG = {
    "psum_bufs": (3, 3, 2),
    "out_copy_split": True,
}


def _get_program():
    if "nc" not in _prog_cache:
        _prog_cache["nc"] = build_program(**BEST_CONFIG)
    return _prog_cache["nc"]


def _get_runner():
    """Build the 8-core PJRT callable once (mirrors bass2jax.run_bass_via_pjrt,
    but cached so repeat kernel() calls skip retrace/relower)."""
    if "runner" in _prog_cache:
        return _prog_cache["runner"]

    import jax
    from jax.sharding import Mesh, PartitionSpec
    from jax.experimental.shard_map import shard_map
    from concourse import mybir as _mybir
    from concourse.bass2jax import (
        _bass_exec_p,
        install_neuronx_cc_hook,
        partition_id_tensor,
    )

    install_neuronx_cc_hook()
    nc = _get_program()
    partition_name = nc.partition_id_tensor.name if nc.partition_id_tensor else None
    in_names, out_names, out_avals, zero_outs = [], [], [], []
    for alloc in nc.m.functions[0].allocations:
        if not isinstance(alloc, _mybir.MemoryLocationSet):
            continue
        name = alloc.memorylocations[0].name
        if alloc.kind == "ExternalInput":
            if name != partition_name:
                in_names.append(name)
        elif alloc.kind == "ExternalOutput":
            out_names.append(name)
            shape = tuple(alloc.tensor_shape)
            dtype = _mybir.dt.np(alloc.dtype)
            out_avals.append(jax.core.ShapedArray(shape, dtype))
            zero_outs.append(np.zeros(shape, dtype))
    n_params = len(in_names)
    all_in = list(in_names) + list(out_names)
    if partition_name is not None:
        all_in.append(partition_name)

    def _body(*args):
        operands = list(args)
        if partition_name is not None:
            operands.append(partition_id_tensor())
        outs = _bass_exec_p.bind(
            *operands,
            out_avals=tuple(out_avals),
            in_names=tuple(all_in),
            out_names=tuple(out_names),
            lowering_input_output_aliases=(),
            sim_require_finite=True,
            sim_require_nnan=True,
            nc=nc,
        )
        return tuple(outs)

    devices = jax.devices()[:NCORES]
    mesh = Mesh(np.asarray(devices), ("core",))
    n_out = len(out_names)
    fn = jax.jit(
        shard_map(
            _body,
            mesh=mesh,
            in_specs=(PartitionSpec("core"),) * (n_params + n_out),
            out_specs=(PartitionSpec("core"),) * n_out,
            check_rep=False,
        ),
        keep_unused=True,
    )
    from jax.sharding import NamedSharding

    shard = NamedSharding(mesh, PartitionSpec("core"))
    concat_zero = [
        jax.device_put(np.zeros((NCORES * z.shape[0], *z.shape[1:]), z.dtype), shard)
        for z in zero_outs
    ]
    runner = (fn, in_names, out_avals, concat_zero)
    _prog_cache["runner"] = runner
    return runner


def kernel(x, Wq, bq, Wk, bk, Wv, bv):
    in_maps = _host_inputs(x, Wq, bq, Wk, bk, Wv, bv)
    fn, in_names, out_avals, concat_zero = _get_runner()
    concat_in = [
        np.concatenate([np.asarray(in_maps[c][nm]) for c in range(NCORES)], axis=0)
        for nm in in_names
    ]
    out_arrs = fn(*concat_in, *concat_zero)
    full = np.asarray(out_arrs[0]).reshape(NCORES * BPC, *out_avals[0].shape[1:])
    return full.astype(np.float32)


# revision 53
# speedup vs baseline: 1.0251x; 1.0194x over previous
"""Trainium2 Bass kernel for nn_AttentionBlock (B=32, C=512, T=1024, key=value=512).

Strategy: data-parallel over batch - each of the 8 NeuronCores processes 4
batches. Per batch, everything stays on-chip. Mixed precision tuned to the
TRN2 cost model.

Key algebraic trick: q.k = x_t^T (Wq^T Wk) x_s + beta_t + alpha_s + c0, so
the host folds Wq and Wk into one matrix M = Wq^T Wk and the device computes
a SINGLE projection g = M x instead of two (q and k). The rank-1 bias terms:
  alpha_s = (Wk^T bq).x_s + bq.bk  -> folds into the ACT exp bias column
  beta_t  = (Wq^T bk).x_t          -> host-broadcast [P,T] tensor added to
                                      the scores PSUM on the Pool engine.

Precision plan (TRN2 cost model: fp8e4 DoubleRow = 0.5 cyc/row over a
256-deep contraction; bf16 = 1 cyc/row):

  g and v projections run in fp8 DoubleRow 3-term (hi*hi + hi*lo + lo*hi)
  from host-packed fp8 hi/lo pairs of x, M, Wv.

  Scores ALSO run in fp8 DoubleRow 3-term: g is split on-device into an
  fp8 hi/lo pair (ACT copy at 1/8 scale + one fused DVE
  scalar_tensor_tensor for the residual), and the moving operand is the
  already-resident fp8 x pack. 3 cyc/column instead of bf16's 4.

  The Pool engine cannot touch PSUM, so beta enters MULTIPLICATIVELY after
  exp: w = exp(xMx*scale + alpha_col) (*) f with f_t = exp(beta_t/sqrt(d))
  host-broadcast; the multiply runs on Pool fused with the row-sum
  (scalar_tensor_tensor accum_out), replacing ACT's accum reads. The
  causal mask is a 0/1 multiply on the diagonal chunk, also on Pool.

  The out matmul runs in bf16 (1 cyc/row at any width) with exact
  128-granular causal skip, as do the exp weights w.

  A PE warmup of dummy matmuls burns the p-state ramp inside the initial
  DMA dead time.

Softmax axis is the QUERY axis (faithful to the reference): row sums Z[s]
along the free axis via activation accum_out, v rows scaled by 1/Z[s].
"""

import math
import os

import numpy as np
import ml_dtypes

os.environ.setdefault("MYCRO_LOCAL_CACHE", "1")

import concourse.bass as bass
from concourse import bacc
import concourse.tile as tile
from concourse import mybir
from concourse.bass_utils import run_bass_kernel_spmd

B, C, T = 32, 512, 1024
KEY = 512
VAL = 512
NCORES = 8
BPC = B // NCORES  # batches per core
P = 128
KT_ = KEY // P     # 4 k-tiles
ST = T // P        # 8 s-tiles
VT = VAL // P      # 4 vd-tiles
SQRT_KEY = math.sqrt(KEY)

SX = 16.0          # host pre-scale on x before fp8 split
SW = 32.0          # host pre-scale on W / M before fp8 split
RS = 16.0          # psum->fp8 rescale divisor for g (SG = SX*SW/RS = 32;
                   # device fp8e4 is IEEE e4m3, max 240 -> |SG*g| must stay low)
SG = SX * SW / RS
DESCALE = 1.0 / (SX * SW)
SCORE_SCALE = 1.0 / (SG * SX * SQRT_KEY)

F32 = mybir.dt.float32
BF16 = mybir.dt.bfloat16
F8 = mybir.dt.float8e4
F16 = mybir.dt.float16
DR = mybir.MatmulPerfMode.DoubleRow

F8NP = mybir.dt.np(F8)     # ml_dtypes.float8_e4m3

WARMUP_N = 60

HI = slice(0, 2)   # packed-level slices along the pair axis
LO = slice(2, 4)
LV = {"hi": HI, "lo": LO}

ALU = mybir.AluOpType


def score_chunks(i):
    """(t0, width, masked) chunks for s-tile i: exact 128-granular causal
    start at t=128*i, chunk widths up to 512 (PSUM bank). The causal
    triangle only touches the first 128 columns of the diagonal chunk."""
    t0 = P * i
    out = []
    masked = True
    while t0 < T:
        wd = min(512, T - t0)
        out.append((t0, wd, masked))
        masked = False
        t0 += wd
    return out


def build_program(
    psum_bufs=(3, 3, 2),
    out_copy_split=True,   # ACT for steady-state out copies, DVE for final
    warmup_n=None,
    dbg=False,
):
    if warmup_n is None:
        warmup_n = WARMUP_N
    nc = bacc.Bacc("TRN2", target_bir_lowering=False, debug=False)

    # packed fp8 hi/lo pairs for DoubleRow:
    #   xpk[b, ch, p, 2*lv + j, t] = fp8(SX * x[b, 256*ch + 128*j + p, t])
    #     split level lv: 0 = hi, 1 = residual
    #   wpk*[ch, p, 2*lv + j, k]   = fp8(SW * W[k, 256*ch + 128*j + p])
    xpk = nc.dram_tensor("xpk", [BPC, 2, P, 4, T], F8, kind="ExternalInput")
    wts_in = {}
    for w in ("m", "v"):
        wts_in[w] = nc.dram_tensor(f"wpk{w}", [2, P, 4, KEY], F8,
                                   kind="ExternalInput")
    # cst packs the 0/1 diag mask [P, 128] (bf16)
    cst = nc.dram_tensor("cst", [P, P], BF16, kind="ExternalInput")
    bvb = nc.dram_tensor("bvb", [P, VAL], F32, kind="ExternalInput")
    # abt[b][p, i] = (alpha[b, 128*i + p] + bq.bk) / sqrt(KEY)
    abt = nc.dram_tensor("abt", [BPC, P, ST], F32, kind="ExternalInput")
    # btb[b][p, t] = exp(beta[b, t] / sqrt(KEY))  (broadcast along partitions)
    btb = nc.dram_tensor("btb", [BPC, P, T], BF16, kind="ExternalInput")
    out = nc.dram_tensor("out", [BPC, VAL, T], F16, kind="ExternalOutput")
    if dbg:
        dbg_t = {
            "gh": nc.dram_tensor("dbg_gh", [P, 2, T], F8, kind="ExternalOutput"),
            "gl": nc.dram_tensor("dbg_gl", [P, 2, T], F8, kind="ExternalOutput"),
            "z": nc.dram_tensor("dbg_z", [P, ST], F32, kind="ExternalOutput"),
            "w": nc.dram_tensor("dbg_w", [P, T], BF16, kind="ExternalOutput"),
            "v": nc.dram_tensor("dbg_v", [P, VAL], BF16, kind="ExternalOutput"),
        }

    with tile.TileContext(nc) as tc:
        with (
            tc.tile_pool(name="const", bufs=1) as cpool,
            tc.tile_pool(name="xp", bufs=6) as xpool,
            tc.tile_pool(name="gp", bufs=16) as gpool,
            tc.tile_pool(name="vp", bufs=16) as vpool,
            tc.tile_pool(name="wtp", bufs=16) as wtpool,
            tc.tile_pool(name="zp", bufs=4) as zpool,
            tc.tile_pool(name="gfp", bufs=3) as gfpool,
            tc.tile_pool(name="abp", bufs=3) as abpool,
            tc.tile_pool(name="btp", bufs=3) as btpool,
            tc.tile_pool(name="osbp", bufs=8) as opool,
            tc.tile_pool(name="psa", bufs=psum_bufs[0], space="PSUM") as psum_a,
            tc.tile_pool(name="pss", bufs=psum_bufs[1], space="PSUM") as psum_s,
            tc.tile_pool(name="pso", bufs=psum_bufs[2], space="PSUM") as psum_o,
        ):
            # ---- constant tiles ----
            wsb = {}
            for w in ("m", "v"):
                for ch in range(2):
                    wsb[w, ch] = cpool.tile(
                        [P, 4, KEY], F8, tag=f"w{w}{ch}", name=f"w{w}{ch}"
                    )
            mask_sb = cpool.tile([P, P], BF16, tag="cst")
            bvb_sb = cpool.tile([P, VAL], F32, tag="bvb")

            def dma_w(w, ch):
                nc.sync.dma_start(wsb[w, ch], wts_in[w][ch])

            # per-batch tile state
            x_t = {}
            ghi_t = {}
            glo_t = {}
            v_t = {}
            wt_t = {}
            zfin_t = {}
            ztmp_t = {}
            rrec_t = {}
            ab_t = {}
            bt_t = {}

            def x_alloc(b):
                if b not in x_t:
                    x_t[b] = [
                        xpool.tile([P, 4, T], F8, tag="x", name=f"x_{b}_{ch}")
                        for ch in range(2)
                    ]

            def x_dma(b, ch, half=None, eng=None):
                x_alloc(b)
                sl = (slice(0, T) if half is None
                      else slice(half * 512, half * 512 + 512))
                (eng or nc.sync).dma_start(
                    x_t[b][ch][:, :, sl], xpk[b, ch][:, :, sl]
                )

            def ab_dma(b):
                ab_t[b] = abpool.tile([P, ST], F32, tag="ab", name=f"ab_{b}")
                bt_t[b] = btpool.tile([P, T], BF16, tag="bt", name=f"bt_{b}")
                nc.sync.dma_start(ab_t[b], abt[b])
                nc.sync.dma_start(bt_t[b], btb[b])

            def phase_x(b):
                x_dma(b, 0)
                x_dma(b, 1)
                ab_dma(b)

            TERMS = (("hi", "hi"), ("hi", "lo"), ("lo", "hi"))

            def g_alloc(b):
                # tiles split by T-half so scores s-tiles 0..3 depend only
                # on the tn=0 projection groups (deps are tile-granular)
                if b not in ghi_t:
                    ghi_t[b] = {
                        (m, h): gpool.tile([P, 2, 512], F8, tag="g",
                                           name=f"gh_{b}_{m}{h}")
                        for m in range(2) for h in range(2)
                    }
                    glo_t[b] = {
                        (m, h): gpool.tile([P, 2, 512], F8, tag="g",
                                           name=f"gl_{b}_{m}{h}")
                        for m in range(2) for h in range(2)
                    }

            def g_move(b, tn, mk, ps):
                """Split one g psum group [128, 512] into fp8 hi+lo pair
                slices at 1/RS scale: hi via ACT copy, lo via one fused
                DVE (psum*1/RS - hi)."""
                m, j = mk // 2, mk % 2
                hi = ghi_t[b][m, tn][:, j, :]
                lo = glo_t[b][m, tn][:, j, :]
                nc.scalar.activation(
                    hi, ps, mybir.ActivationFunctionType.Copy,
                    scale=1.0 / RS,
                )
                nc.vector.scalar_tensor_tensor(
                    out=lo, in0=ps, scalar=1.0 / RS, in1=hi,
                    op0=ALU.mult, op1=ALU.subtract,
                )

            def phase_g(b):
                """g = M x projection: [128c', T] tiles packed as fp8 hi/lo
                DoubleRow pairs via fp8 triple DoubleRow matmuls. Groups
                alternate psum_a/psum_s banks (scores' banks are idle during
                the projection) so bank-release latency never stalls PE."""
                xs = x_t[b]
                g_alloc(b)
                for tn in range(2):
                    for mk in range(KT_):
                        if (tn * KT_ + mk) % 2:
                            ps = psum_s.tile([P, 512], F32, tag="ps_s",
                                             name="ps_s")
                        else:
                            ps = psum_a.tile([P, 512], F32, tag="ps_a",
                                             name="ps_a")
                        idx = 0
                        for wl, xl in TERMS:
                            for ch in range(2):
                                nc.tensor.matmul(
                                    ps,
                                    wsb["m", ch][:, LV[wl], mk * P:(mk + 1) * P],
                                    xs[ch][:, LV[xl], tn * 512:(tn + 1) * 512],
                                    start=(idx == 0),
                                    stop=(idx == 5),
                                    perf_mode=DR,
                                )
                                idx += 1
                        g_move(b, tn, mk, ps)

            def phase_v_group(b, ti):
                """One v[t, vd] tile [128t, VAL] bf16 at (SX*SW) scale; bias
                added by the DVE move (the scale folds out in the out-copy)."""
                xs = x_t[b]
                if b not in v_t:
                    v_t[b] = [
                        vpool.tile([P, VAL], BF16, tag="v", name=f"v_{b}_{i}")
                        for i in range(ST)
                    ]
                # alternate psum_o/psum_a (out banks are idle during v)
                if ti % 2 == 0:
                    ps = psum_o.tile([P, 512], F32, tag="ps_o", name="ps_o")
                else:
                    ps = psum_a.tile([P, 512], F32, tag="ps_a", name="ps_a")
                idx = 0
                for xl, wl in TERMS:
                    for ch in range(2):
                        nc.tensor.matmul(
                            ps,
                            xs[ch][:, LV[xl], ti * P:(ti + 1) * P],
                            wsb["v", ch][:, LV[wl], :],
                            start=(idx == 0),
                            stop=(idx == 5),
                            perf_mode=DR,
                        )
                        idx += 1
                nc.vector.tensor_add(out=v_t[b][ti], in0=ps, in1=bvb_sb)

            def phase_v(b):
                for ti in range(ST):
                    phase_v_group(b, ti)

            def phase_b(b, ilo=0, ihi=ST):
                """scoresT[s,t] per (s-tile, chunk) with exact causal start,
                fp8 DoubleRow from the g pair and the resident x pack; beta
                broadcast-add + causal mask on Pool; exp on ACT with
                accum_out row sums and the alpha bias column."""
                xs = x_t[b]
                if b not in wt_t:
                    wt_t[b] = [
                        wtpool.tile([P, T], BF16, tag="wt", name=f"wt_{b}_{i}")
                        for i in range(ST)
                    ]
                    zfin_t[b] = zpool.tile([P, ST], F32, tag="zfin", name="zfin")
                    ztmp_t[b] = zpool.tile([P, 2 * ST], F32, tag="ztmp", name="ztmp")
                wts = wt_t[b]
                zfin, ztmp = zfin_t[b], ztmp_t[b]
                ghi, glo = ghi_t[b], glo_t[b]

                for i in range(ilo, ihi):
                    chunks = score_chunks(i)
                    n = len(chunks)
                    ih, io = i // 4, (i % 4) * P
                    for ci, (t0, wd, masked) in enumerate(chunks):
                        ps = psum_s.tile([P, 512], F32, tag="ps_s", name="ps_s")
                        idx = 0
                        for gl, xl in TERMS:
                            gt = ghi if gl == "hi" else glo
                            for m in range(2):
                                nc.tensor.matmul(
                                    ps[:, 0:wd],
                                    gt[m, ih][:, :, io:io + P],
                                    xs[m][:, LV[xl], t0:t0 + wd],
                                    start=(idx == 0),
                                    stop=(idx == 5),
                                    perf_mode=DR,
                                )
                                idx += 1
                        wsl = wts[i][:, t0:t0 + wd]
                        nc.scalar.activation(
                            wsl,
                            ps[:, 0:wd],
                            mybir.ActivationFunctionType.Exp,
                            bias=ab_t[b][:, i:i + 1],
                            scale=SCORE_SCALE,
                        )
                        if masked:
                            # zero the sub-causal triangle of the diagonal
                            # 128-block (0/1 mask, SBUF-only so Pool is ok)
                            nc.gpsimd.tensor_mul(
                                out=wts[i][:, t0:t0 + P],
                                in0=wts[i][:, t0:t0 + P],
                                in1=mask_sb,
                            )
                        zcol = (zfin[:, i:i + 1] if n == 1
                                else ztmp[:, 2 * i + ci:2 * i + ci + 1])
                        # w = e0 * f (beta factor) fused with the row sum
                        # (TensorScalarPtr is rejected on Pool, so DVE)
                        nc.vector.scalar_tensor_tensor(
                            out=wsl, in0=wsl, scalar=1.0,
                            in1=bt_t[b][:, t0:t0 + wd],
                            op0=ALU.mult, op1=ALU.mult,
                            accum_out=zcol,
                        )
                    if n > 1:
                        nc.vector.tensor_add(
                            out=zfin[:, i:i + 1],
                            in0=ztmp[:, 2 * i:2 * i + 1],
                            in1=ztmp[:, 2 * i + 1:2 * i + 2],
                        )

            def rrec_alloc(b):
                if b not in rrec_t:
                    rrec_t[b] = zpool.tile([P, ST], F32, tag="rrec", name="rrec")
                return rrec_t[b]

            def phase_c_tile(b, ti):
                """1/Z + scale v row for a single s-tile (fine-grained so out
                groups never wait on a whole half)."""
                rrec = rrec_alloc(b)
                nc.vector.reciprocal(
                    rrec[:, ti:ti + 1], zfin_t[b][:, ti:ti + 1]
                )
                nc.vector.tensor_scalar_mul(
                    v_t[b][ti], v_t[b][ti], rrec[:, ti:ti + 1]
                )

            def phase_c_half(b, h):
                rrec = rrec_alloc(b)
                nc.vector.reciprocal(
                    rrec[:, 4 * h:4 * h + 4], zfin_t[b][:, 4 * h:4 * h + 4]
                )
                for ti in range(4 * h, 4 * h + 4):
                    nc.vector.tensor_scalar_mul(
                        v_t[b][ti], v_t[b][ti], rrec[:, ti:ti + 1]
                    )

            def emit_d_group(b, c, vm, f0=0, f1=512, copy_eng="act",
                             pool=None, dma_eng=None):
                """One out-psum group for (t-chunk c, vd-tile vm), columns
                [f0,f1) of the chunk. w rows are defined-zero above their
                causal start, so full-width matmuls are valid; tiles whose
                causal start lies inside the chunk get width slices."""
                vs, wts = v_t[b], wt_t[b]
                if pool is None:
                    ps = psum_o.tile([P, 512], F32, tag="ps_o", name="ps_o")
                else:
                    ps = pool[0].tile([P, 512], F32, tag=pool[1], name=pool[1])
                t0g = 512 * c + f0
                t1g = 512 * c + f1
                scs = [sc for sc in range(4 * (c + 1)) if P * sc < t1g]
                for idx, sc in enumerate(scs):
                    lo = max(t0g, P * sc)
                    nc.tensor.matmul(
                        ps[:, lo - 512 * c:f1],
                        vs[sc][:, vm * P:(vm + 1) * P],
                        wts[sc][:, lo:t1g],
                        start=(idx == 0),
                        stop=(idx == len(scs) - 1),
                    )
                oslice = out[b, vm * P:(vm + 1) * P, t0g:t1g]
                psl = ps[:, f0:f1] if (f1 - f0) != 512 else ps
                osb = opool.tile([P, 512], F16, tag="osb", name="osb")
                osl = osb[:, 0:f1 - f0] if (f1 - f0) != 512 else osb
                if copy_eng == "act" and out_copy_split:
                    nc.scalar.activation(
                        osl, psl, mybir.ActivationFunctionType.Copy,
                        scale=DESCALE,
                    )
                else:
                    nc.vector.tensor_scalar_mul(osl, psl, DESCALE)
                (dma_eng or nc.sync).dma_start(oslice, osl)

            def phase_d_half_group(b, h, vm, steady_eng="act"):
                """Single steady-state out group (for interleaving with
                score tiles)."""
                if steady_eng == "mix":
                    eng = "dve" if vm % 2 else "act"
                else:
                    eng = steady_eng
                pool = (psum_a, "ps_a") if vm % 2 else None
                emit_d_group(b, h, vm, copy_eng=eng, pool=pool)

            def phase_d_half(b, h, final=False, steady_eng="act"):
                for vm in range(VT):
                    if final:
                        # scores done: borrow psum_s banks, alternate copy
                        # engines + DMA queues so the drain parallelizes
                        eng = "dve" if vm % 2 else "act"
                        pool = (psum_s, "ps_s") if vm % 2 else None
                        dq = nc.scalar if vm % 2 else nc.sync
                    else:
                        # steady state: ACT does the copies by default (DVE
                        # carries the f-pass and v ops); borrow idle psum_a
                        # banks so groups never wait on copy drain
                        if steady_eng == "mix":
                            eng = "dve" if vm % 2 else "act"
                        else:
                            eng = steady_eng
                        pool = (psum_a, "ps_a") if vm % 2 else None
                        dq = None
                    if final and vm == VT - 1:
                        oeng = "act" if eng == "dve" else "dve"
                        odq = nc.sync if vm % 2 else nc.scalar
                        emit_d_group(b, h, vm, 0, 256, copy_eng=eng, pool=pool,
                                     dma_eng=dq)
                        emit_d_group(b, h, vm, 256, 512, copy_eng=oeng,
                                     dma_eng=odq)
                    else:
                        emit_d_group(b, h, vm, copy_eng=eng, pool=pool,
                                     dma_eng=dq)

            def phase_cd(b):
                for h in range(2):
                    phase_c_half(b, h)
                    phase_d_half(b, h)

            # ---- startup ----
            def proj_g0_waves():
                """Whole g projection in two tn-major waves of 4 psum banks
                each (borrowed across pools -- all idle this early), ch0
                terms first within each wave. Wave A (t 0:512) starts after
                just M-ch0 + x-ch0-h0; wave B (t 512:1024) starts once the
                h1 DMAs land."""
                xs = x_t[0]
                g_alloc(0)
                terms6 = [(wl, xl, ch) for ch in range(2) for wl, xl in TERMS]
                wave_pools = {
                    0: [(psum_a, "ps_a")] * 3 + [(psum_s, "ps_s")],
                    1: [(psum_s, "ps_s")] * 2 + [(psum_o, "ps_o")] * 2,
                }
                for tn in range(2):
                    pss = {}
                    for mk in range(KT_):
                        pool, tag = wave_pools[tn][mk]
                        pss[mk] = pool.tile([P, 512], F32, tag=tag, name=tag)
                    for idx, (wl, xl, ch) in enumerate(terms6):
                        for mk in range(KT_):
                            nc.tensor.matmul(
                                pss[mk],
                                wsb["m", ch][:, LV[wl], mk * P:(mk + 1) * P],
                                xs[ch][:, LV[xl], tn * 512:(tn + 1) * 512],
                                start=(idx == 0),
                                stop=(idx == 5),
                                perf_mode=DR,
                            )
                    for mk in range(KT_):
                        g_move(0, tn, mk, pss[mk])

            def pe_warmup(n):
                """Burn the PE p-state ramp (the cost model runs matmuls at
                half clock for the first ~3us of PE activity) on dummy
                matmuls while the first DMAs are in flight."""
                warm = cpool.tile([P, 64], BF16, tag="warm", name="warm")
                nc.gpsimd.memset(warm, 1.0)
                wps = psum_o.tile([P, 512], F32, tag="ps_o", name="ps_o")
                for _ in range(n):
                    nc.tensor.matmul(
                        wps[:, 0:64][0:64, :], warm, warm, start=True,
                        stop=True,
                    )

            def startup():
                pe_warmup(warmup_n)
                x_alloc(0)
                # split the critical startup feed across both HWDGE queues:
                # sync carries weights + x-h1, scalar carries x-h0 halves,
                # so the first matmul's two operands arrive in parallel
                dma_w("m", 0)
                x_dma(0, 0, 0, eng=nc.scalar)
                dma_w("m", 1)
                x_dma(0, 1, 0, eng=nc.scalar)
                x_dma(0, 0, 1)
                x_dma(0, 1, 1, eng=nc.scalar)
                dma_w("v", 0)
                proj_g0_waves()
                dma_w("v", 1)
                # constants are needed later (scores(0) epilogue / v bias);
                # emit after g0 so they don't compete with wv in the pipe
                ab_t[0] = abpool.tile([P, ST], F32, tag="ab", name="ab_0")
                bt_t[0] = btpool.tile([P, T], BF16, tag="bt", name="bt_0")
                nc.scalar.dma_start(mask_sb, cst[:, :])
                nc.scalar.dma_start(bvb_sb, bvb[:, :])
                nc.scalar.dma_start(ab_t[0], abt[0])
                nc.scalar.dma_start(bt_t[0], btb[0])

            startup()
            # interleave v(0) groups with scores(0) tiles so ACT's exp
            # burst overlaps the v projection instead of stalling PE.
            # v first: wv lands before the wave-A g moves complete.
            for g8 in range(ST):
                phase_v_group(0, g8)
                phase_b(0, g8, g8 + 1)
            phase_c_half(0, 0)
            phase_c_half(0, 1)
            phase_x(1)
            for b in range(1, BPC):
                phase_g(b)
                phase_v(b)
                if dbg and b == 1:
                    for h_ in range(2):
                        sl_ = slice(h_ * 512, (h_ + 1) * 512)
                        nc.sync.dma_start(dbg_t["gh"][:, :, sl_],
                                          ghi_t[1][0, h_])
                        nc.sync.dma_start(dbg_t["gl"][:, :, sl_],
                                          glo_t[1][0, h_])
                    nc.sync.dma_start(dbg_t["v"][:], v_t[1][2])
                if b + 1 < BPC:
                    phase_x(b + 1)
                if b == BPC - 1:
                    # tail: interleave b-1's out groups with b's score tiles
                    # (alternating copy engines), then a progressive drain:
                    # the final half's columns [512,896) only need s-tiles
                    # 0..6, so those pieces start before s7's scores land
                    for g8 in range(8):
                        phase_d_half_group(b - 1, g8 // 4, g8 % 4,
                                           steady_eng="mix")
                        phase_b(b, g8 // 2, g8 // 2 + 1) if g8 % 2 == 0 \
                            else None
                    phase_c_half(b, 0)
                    for vm in range(VT):
                        emit_d_group(b, 0, vm, copy_eng="dve")
                        phase_b(b, 4 + vm, 5 + vm)
                        phase_c_tile(b, 4 + vm)
                    phase_d_half(b, 1, final=True)
                else:
                    # interleave b-1's 8 out groups with b's 8 score tiles:
                    # spreads ACT's exp burst across the whole PE window
                    for g8 in range(8):
                        phase_d_half_group(b - 1, g8 // 4, g8 % 4)
                        phase_b(b, g8, g8 + 1)
                    phase_c_half(b, 0)
                    phase_c_half(b, 1)
                    if dbg and b == 1:
                        nc.sync.dma_start(dbg_t["z"][:], zfin_t[1])
                        nc.sync.dma_start(dbg_t["w"][:], wt_t[1][2])

    nc.compile()
    return nc


def _split8(a, scale):
    """Split scale*a into fp8 hi + fp8 residual (both as fp8 arrays)."""
    a = np.asarray(a, dtype=np.float32) * scale
    hi = a.astype(F8NP)
    lo = (a - hi.astype(np.float32)).astype(F8NP)
    return hi, lo


def _pack_w(Wt, scale):
    """W [K, C] -> fp8 [2(ch), 128(p), 4(2*lv+j), K] with
    value[ch,p,2*lv+j,k] = fp8_lv(scale * W[k, 256*ch + 128*j + p])."""
    K = Wt.shape[0]
    hi, lo = _split8(Wt, scale)            # [K, C] fp8
    def lay(a):
        # [K, C] -> [ch, j, p, K] -> [ch, p, j, K]
        return a.T.reshape(2, 2, P, K).transpose(0, 2, 1, 3)
    hi, lo = lay(hi), lay(lo)              # [2, P, 2, K]
    pk = np.concatenate([hi, lo], axis=2)  # [2, P, 4, K]
    return np.ascontiguousarray(pk)


def _pack_x(xb, scale):
    """x [BPC, C, T] -> fp8 [BPC, 2(ch), 128(p), 4(2*lv+j), T]."""
    hi, lo = _split8(xb, scale)            # [BPC, C, T] fp8
    def lay(a):
        return a.reshape(BPC, 2, 2, P, T).transpose(0, 1, 3, 2, 4)
    pk = np.concatenate([lay(hi), lay(lo)], axis=3)  # [BPC, 2, P, 4, T]
    return np.ascontiguousarray(pk)


def _host_inputs(x, Wq, bq, Wk, bk, Wv, bv):
    x = np.asarray(x, dtype=np.float32)
    Wq = np.asarray(Wq, dtype=np.float32)
    Wk = np.asarray(Wk, dtype=np.float32)
    bq = np.asarray(bq, dtype=np.float32)
    bk = np.asarray(bk, dtype=np.float32)
    bv = np.asarray(bv, dtype=np.float32)

    M = Wq.T @ Wk                          # [C(c_t), C(c_s)]
    ua = Wk.T @ bq                         # alpha_s = ua . x_s + c0
    ub = Wq.T @ bk                         # beta_t  = ub . x_t
    c0 = float(bq @ bk)

    w8 = {
        "wpkm": _pack_w(M, SW),
        "wpkv": _pack_w(np.asarray(Wv, dtype=np.float32), SW),
    }

    r = np.arange(P)[:, None]
    u = np.arange(P)[None, :]
    maskd = np.where(u >= r, 1.0, 0.0).astype(ml_dtypes.bfloat16)
    cstv = np.ascontiguousarray(maskd)
    bvbv = np.ascontiguousarray(
        np.broadcast_to((bv * (SX * SW))[None, :], (P, VAL)).copy()
    )

    in_maps = []
    for c in range(NCORES):
        xb = x[c * BPC:(c + 1) * BPC]                       # [BPC, C, T]
        alpha = np.einsum("c,bct->bt", ua, xb) + c0         # [BPC, T]
        beta = np.einsum("c,bct->bt", ub, xb)               # [BPC, T]
        abtv = np.ascontiguousarray(
            (alpha / SQRT_KEY).reshape(BPC, ST, P).transpose(0, 2, 1)
        ).astype(np.float32)
        btbv = np.ascontiguousarray(np.broadcast_to(
            np.exp(beta / SQRT_KEY)[:, None, :], (BPC, P, T)
        )).astype(ml_dtypes.bfloat16)
        m = {
            "xpk": _pack_x(xb, SX),
            "cst": cstv, "bvb": bvbv,
            "abt": abtv, "btb": btbv,
        }
        m.update(w8)
        in_maps.append(m)
    return in_maps


_prog_cache = {}


BEST_CONFIG = {
    "psum_bufs": (3, 3, 2),
    "out_copy_split": True,
}


def _get_program():
    if "nc" not in _prog_cache:
        _prog_cache["nc"] = build_program(**BEST_CONFIG)
    return _prog_cache["nc"]


def _get_runner():
    """Build the 8-core PJRT callable once (mirrors bass2jax.run_bass_via_pjrt,
    but cached so repeat kernel() calls skip retrace/relower)."""
    if "runner" in _prog_cache:
        return _prog_cache["runner"]

    import jax
    from jax.sharding import Mesh, PartitionSpec
    from jax.experimental.shard_map import shard_map
    from concourse import mybir as _mybir
    from concourse.bass2jax import (
        _bass_exec_p,
        install_neuronx_cc_hook,
        partition_id_tensor,
    )

    install_neuronx_cc_hook()
    nc = _get_program()
    partition_name = nc.partition_id_tensor.name if nc.partition_id_tensor else None
    in_names, out_names, out_avals, zero_outs = [], [], [], []
    for alloc in nc.m.functions[0].allocations:
        if not isinstance(alloc, _mybir.MemoryLocationSet):
            continue
        name = alloc.memorylocations[0].name
        if alloc.kind == "ExternalInput":
            if name != partition_name:
                in_names.append(name)
        elif alloc.kind == "ExternalOutput":
            out_names.append(name)
            shape = tuple(alloc.tensor_shape)
            dtype = _mybir.dt.np(alloc.dtype)
            out_avals.append(jax.core.ShapedArray(shape, dtype))
            zero_outs.append(np.zeros(shape, dtype))
    n_params = len(in_names)
    all_in = list(in_names) + list(out_names)
    if partition_name is not None:
        all_in.append(partition_name)

    def _body(*args):
        operands = list(args)
        if partition_name is not None:
            operands.append(partition_id_tensor())
        outs = _bass_exec_p.bind(
            *operands,
            out_avals=tuple(out_avals),
            in_names=tuple(all_in),
            out_names=tuple(out_names),
            lowering_input_output_aliases=(),
            sim_require_finite=True,
            sim_require_nnan=True,
            nc=nc,
        )
        return tuple(outs)

    devices = jax.devices()[:NCORES]
    mesh = Mesh(np.asarray(devices), ("core",))
    n_out = len(out_names)
    fn = jax.jit(
        shard_map(
            _body,
            mesh=mesh,
            in_specs=(PartitionSpec("core"),) * (n_params + n_out),
            out_specs=(PartitionSpec("core"),) * n_out,
            check_rep=False,
        ),
        keep_unused=True,
    )
    from jax.sharding import NamedSharding

    shard = NamedSharding(mesh, PartitionSpec("core"))
    concat_zero = [
        jax.device_put(np.zeros((NCORES * z.shape[0], *z.shape[1:]), z.dtype), shard)
        for z in zero_outs
    ]
    runner = (fn, in_names, out_avals, concat_zero)
    _prog_cache["runner"] = runner
    return runner


def kernel(x, Wq, bq, Wk, bk, Wv, bv):
    in_maps = _host_inputs(x, Wq, bq, Wk, bk, Wv, bv)
    fn, in_names, out_avals, concat_zero = _get_runner()
    concat_in = [
        np.concatenate([np.asarray(in_maps[c][nm]) for c in range(NCORES)], axis=0)
        for nm in in_names
    ]
    out_arrs = fn(*concat_in, *concat_zero)
    full = np.asarray(out_arrs[0]).reshape(NCORES * BPC, *out_avals[0].shape[1:])
    return full.astype(np.float32)
